# revision 1
# baseline (speedup 1.0000x reference)
"""Trainium2 Bass kernel for the conv(k=2, paired-with-t0) -> FC1 -> FC2 model.

Model (see reference):
  x [B=8192, 5661] -> view [B, 111, 51]
  y[b,t,o] = relu( sum_c Wc[o,c,0]*x[b,0,c] + Wc[o,c,1]*x[b,1+t,c] + bc[o] )
  flat channel-major y[b, o*110+t] -> h = relu(y @ W1.T + b1) -> out = h @ W2.T + b2

Strategy: pure data parallel over the batch across 8 NeuronCores (1024 rows
per core). On each core, per batch block of 512 rows and per timestep t:
  - conv is a K=51 matmul with the 51-channel contraction on partitions;
    the two 128-channel output chunks run concurrently in disjoint PE
    row-strips (rows 0..51 / 64..115) via tile_position row tiling.
  - the timestep-0 contribution + conv bias enter the same PSUM accumulation
    through an augmented-K (52) matmul with a constant ones row in x.
  - relu + bf16 cast: one chunk on ScalarE, one on VectorE.
  - FC1 accumulates all 110 timesteps into 4 PSUM banks ([128 batch, 400]);
    b1 enters via a K=1 ones-row matmul at accumulation start.
  - FC2: PE-transpose h to [f, b], then 4 accumulating matmuls; b2 via
    ScalarE bias.
Host side: shard/transpose x, pre-pack weights, gather [2, 1024] outputs.
"""

import os
import sys

if "/opt/trn_rl_repo" not in sys.path:
    sys.path.insert(0, "/opt/trn_rl_repo")

import numpy as np
import ml_dtypes

CL = 111          # context length
IL = 51           # inst length (conv channels in)
PC = 256          # conv channels out
F1 = 400          # fc1 width
OUT = 2           # fc2 width
B = 8192          # batch
NCORES = 8
BC = B // NCORES  # 1024 rows per core
BLK = 512         # batch block (matmul moving free dim)
NBLK = BC // BLK  # 2
NT = CL - 1       # 110 timesteps

BF16 = ml_dtypes.bfloat16

_CACHE = {}


def _build_nc(reps=1, ablate=(), loop_n=0):
    """Build + compile the per-core Bass program (same NEFF on all cores).

    reps>1 repeats the whole body (for on-device timing via slope);
    ablate: subset of {"w1dma", "xdma", "fc1", "conv", "relu"} for
    bottleneck experiments (output becomes wrong).
    """
    key = ("nc", reps, tuple(sorted(ablate)), loop_n)
    if key in _CACHE:
        return _CACHE[key]

    import concourse.bass as bass
    import concourse.bacc as bacc
    import concourse.mybir as mybir
    import concourse.tile as tile
    from concourse import masks

    DT = mybir.dt.bfloat16
    F32 = mybir.dt.float32
    RELU = mybir.ActivationFunctionType.Relu

    nc = bacc.Bacc("TRN2", target_bir_lowering=False, debug=False,
                   num_devices=NCORES)

    TC = 11                    # timesteps per DMA chunk
    NCH = NT // TC             # 10 chunks
    xh_d = nc.dram_tensor("xh", (NBLK, 128, CL, BLK), DT, kind="ExternalInput").ap()
    w1_d = nc.dram_tensor("w1h", (128, NT, 800), DT, kind="ExternalInput").ap()
    wc1_d = nc.dram_tensor("wc1p", (128, 128), DT, kind="ExternalInput").ap()
    wc0_d = nc.dram_tensor("wc0p", (128, 128), DT, kind="ExternalInput").ap()
    b1_d = nc.dram_tensor("b1r", (1, F1), DT, kind="ExternalInput").ap()
    w2_d = nc.dram_tensor("w2tp", (128, 8), DT, kind="ExternalInput").ap()
    b2_d = nc.dram_tensor("b2c", (OUT, 1), F32, kind="ExternalInput").ap()
    o_d = nc.dram_tensor("o", (OUT, BC), F32, kind="ExternalOutput").ap()

    with tile.TileContext(nc) as tc:
        with (
            tc.tile_pool(name="const", bufs=1) as cpool,
            tc.tile_pool(name="stream", bufs=3) as spool,
            tc.tile_pool(name="psum", bufs=1, space="PSUM") as ppool,
        ):
            wc1 = cpool.tile([128, 128], DT)
            nc.sync.dma_start(wc1[:], wc1_d)
            wc0 = cpool.tile([128, 128], DT)
            nc.sync.dma_start(wc0[:], wc0_d)
            w2t = cpool.tile([128, 8], DT)
            nc.sync.dma_start(w2t[:], w2_d)
            b1r = cpool.tile([1, F1], DT)
            nc.sync.dma_start(b1r[:], b1_d)
            b2c = cpool.tile([OUT, 1], F32)
            nc.sync.dma_start(b2c[:], b2_d)
            ones = cpool.tile([1, 128], DT)
            nc.vector.memset(ones[:], 1.0)
            ident = cpool.tile([128, 128], DT)
            masks.make_identity(nc, ident[:])

            ysb_res = None
            if "relu" in ablate:
                ysb_res = [cpool.tile([128, TC, BLK], DT, name=f"ysb_res{c}")
                           for c in range(2)]
                for c in range(2):
                    nc.vector.memset(ysb_res[c][:], 0.01)

            import contextlib
            loop_cm = tc.For_i(0, loop_n, 1) if loop_n else contextlib.nullcontext()
            with loop_cm:
             for rep in range(reps):
              for blk in range(NBLK):
                u = f"{rep}_{blk}"
                # x timestep 0 (channels + ones row at partition 51/115,
                # strip duplicate baked in rows 64..115), block-resident
                xt0 = spool.tile([128, BLK], DT, tag="xt0", bufs=2, name=f"xt0_{u}")
                if "xdma" not in ablate:
                    nc.scalar.dma_start(xt0[:], xh_d[blk, :, 0, :])

                # rotating conv-output PSUM tiles (2 strips x 2-deep)
                ypool = [
                    ppool.tile([128, BLK], F32, tag=f"yr{i}", bufs=1,
                               name=f"yr{u}_{i}")
                    for i in range(4)
                ]
                # fc1 accumulators, one per 128-row batch subtile
                hps = [
                    ppool.tile([128, F1], F32, tag="h", bufs=4, name=f"hps{u}_{j}")
                    for j in range(4)
                ]
                # b1 bias enters the accumulation via K=1 ones matmul
                for j in range(4):
                    nc.tensor.matmul(hps[j][:], ones[:], b1r[:],
                                     start=True, stop=False)

                # chunk tile getter: allocates stream tiles + DMAs on first use
                chunk_tiles = {}

                def get_chunk(ch, u=u, blk=blk, spool=spool, chunk_tiles=chunk_tiles):
                    if ch in chunk_tiles:
                        return chunk_tiles[ch]
                    xc = spool.tile([128, TC, BLK], DT, tag="xc", name=f"xc{u}_{ch}")
                    if "xdma" not in ablate:
                        nc.scalar.dma_start(
                            xc[:], xh_d[blk, :, 1 + ch * TC:1 + (ch + 1) * TC, :])
                    w1c = spool.tile([128, TC, 800], DT, tag="w1c", name=f"w1c{u}_{ch}")
                    if "w1dma" not in ablate:
                        nc.sync.dma_start(
                            w1c[:], w1_d[:, ch * TC:(ch + 1) * TC, :])
                    ysb0c = spool.tile([128, TC, BLK], DT, tag="ysb0", bufs=2,
                                       name=f"ysb0c{u}_{ch}")
                    ysb1c = spool.tile([128, TC, BLK], DT, tag="ysb1", bufs=2,
                                       name=f"ysb1c{u}_{ch}")
                    chunk_tiles[ch] = (xc, w1c, ysb0c, ysb1c)
                    return chunk_tiles[ch]

                def conv_c0(t):
                    y0 = ypool[2 * (t % 2)]
                    y1 = ypool[2 * (t % 2) + 1]
                    if "c0" not in ablate:
                        nc.tensor.matmul(y0[:], wc0[0:52, :], xt0[0:52, :],
                                         start=True, stop=False)
                        nc.tensor.matmul(y1[:], wc0[64:116, :], xt0[64:116, :],
                                         start=True, stop=False,
                                         tile_position=(64, 0))

                def conv_ci(t):
                    xc = get_chunk(t // TC)[0]
                    k = t % TC
                    y0 = ypool[2 * (t % 2)]
                    y1 = ypool[2 * (t % 2) + 1]
                    c0_on = "c0" not in ablate
                    nc.tensor.matmul(y0[:], wc1[0:51, :], xc[0:51, k, :],
                                     start=not c0_on, stop=True)
                    nc.tensor.matmul(y1[:], wc1[64:115, :], xc[64:115, k, :],
                                     start=not c0_on, stop=True,
                                     tile_position=(64, 0))

                def conv(t):
                    conv_c0(t)
                    conv_ci(t)

                # software pipeline: conv one timestep ahead of relu/fc1
                if "conv" not in ablate:
                    conv(0)
                for t in range(NT):
                    _, w1c, ysb0c, ysb1c = get_chunk(t // TC)
                    k = t % TC
                    y0 = ypool[2 * (t % 2)]
                    y1 = ypool[2 * (t % 2) + 1]
                    if "relu" in ablate:
                        if "conv" not in ablate:
                            nc.scalar.activation(ysb_res[0][:, k, :], y0[:], RELU)
                            nc.vector.tensor_relu(ysb_res[1][:, k, :], y1[:])
                    else:
                        nc.scalar.activation(ysb0c[:, k, :], y0[:], RELU)
                        nc.vector.tensor_relu(ysb1c[:, k, :], y1[:])
                    if "conv" not in ablate and t + 1 < NT:
                        conv_c0(t + 1)
                    last = t == NT - 1
                    if "fc1" not in ablate:
                        pair = (((0, ysb_res[0]), (1, ysb_res[1]))
                                if "relu" in ablate else
                                ((0, ysb0c), (1, ysb1c)))
                        kk = (k + 5) % TC if "relu" in ablate else k
                        same_w = "samew" in ablate
                        for c, ysbc in pair:
                            for j in range(4):
                                nc.tensor.matmul(
                                    hps[j][:],
                                    ysbc[:, 0, 0:128] if same_w else
                                    ysbc[:, kk, j * 128:(j + 1) * 128],
                                    w1c[:, k, c * F1:(c + 1) * F1],
                                    start=False, stop=(last and c == 1),
                                )
                            if c == 0 and "conv" not in ablate and t + 1 < NT:
                                conv_ci(t + 1)

                # ---- tail: h relu, transpose to [f, b], fc2 ----
                hsb = []
                for j in range(4):
                    hsbj = spool.tile([128, F1], DT, tag="hsb", bufs=4,
                                      name=f"hsb{u}_{j}")
                    nc.scalar.activation(hsbj[:], hps[j][:], RELU)
                    hsb.append(hsbj)

                outp = ppool.tile([OUT, BLK], F32, tag="h", bufs=4,
                                  name=f"outp_{u}")
                for fc in range(4):
                    w = 128 if fc < 3 else F1 - 3 * 128
                    hTp = ppool.tile([128, BLK], DT, tag="h", bufs=4,
                                     name=f"hTp_{u}_{fc}")
                    for j in range(4):
                        nc.tensor.transpose(
                            hTp[0:w, j * 128:(j + 1) * 128],
                            hsb[j][:, fc * 128:fc * 128 + w],
                            ident[:],
                        )
                    hTs = spool.tile([128, BLK], DT, tag="hTs", bufs=2,
                                     name=f"hTs_{u}_{fc}")
                    nc.vector.tensor_copy(hTs[0:w, :], hTp[0:w, :])
                    nc.tensor.matmul(outp[:], w2t[0:w, 2 * fc:2 * fc + 2],
                                     hTs[0:w, :],
                                     start=(fc == 0), stop=(fc == 3))

                osb = spool.tile([OUT, BLK], F32, tag="osb", bufs=2,
                                 name=f"osb_{u}")
                nc.scalar.add(osb[:], outp[:], b2c[:])
                nc.sync.dma_start(o_d[:, blk * BLK:(blk + 1) * BLK], osb[:])

    nc.compile()
    _CACHE[key] = nc
    return nc


def _host_prep(x, Wc, bc, W1, b1, W2, b2):
    """Shard + lay out inputs for the per-core Bass program."""
    x = np.asarray(x, dtype=np.float32)
    Wc = np.asarray(Wc, dtype=np.float32)
    bc = np.asarray(bc, dtype=np.float32)
    W1 = np.asarray(W1, dtype=np.float32)
    b1 = np.asarray(b1, dtype=np.float32)
    W2 = np.asarray(W2, dtype=np.float32)
    b2 = np.asarray(b2, dtype=np.float32)

    # x -> [core, block, partition-row, t, batch-within-block]
    # rows 0..50 = channels, 51 = ones, 52..63 = 0, 64..115 = strip dup, 116..127 = 0
    A = (x.reshape(NCORES, NBLK, BLK, CL, IL)
         .transpose(0, 1, 4, 3, 2)          # [8, 2, 51, 111, 512]
         .astype(BF16))
    xh = np.zeros((NCORES, NBLK, 128, CL, BLK), dtype=BF16)
    xh[:, :, 0:51] = A
    xh[:, :, 51] = np.ones((1,), dtype=BF16)
    xh[:, :, 64:115] = A
    xh[:, :, 115] = np.ones((1,), dtype=BF16)

    # conv weights packed into the two PE row strips
    wc1p = np.zeros((128, 128), dtype=np.float32)
    wc1p[0:51, :] = Wc[:128, :, 1].T
    wc1p[64:115, :] = Wc[128:, :, 1].T
    wc0p = np.zeros((128, 128), dtype=np.float32)
    wc0p[0:51, :] = Wc[:128, :, 0].T
    wc0p[51, :] = bc[:128]
    wc0p[64:115, :] = Wc[128:, :, 0].T
    wc0p[115, :] = bc[128:]

    # W1 -> [partition(o within chunk), t, chunk*400 + f]  (t contiguous per
    # partition so one DMA covers many timesteps contiguously)
    w1h = np.ascontiguousarray(
        W1.reshape(F1, PC, NT).transpose(2, 1, 0)      # [110, 256, 400]
        .reshape(NT, 2, 128, F1).transpose(2, 0, 1, 3)  # [128, 110, 2, 400]
        .reshape(128, NT, 800)
    ).astype(BF16)

    w2tp = np.zeros((128, 8), dtype=np.float32)
    for fc in range(4):
        w = min(128, F1 - fc * 128)
        w2tp[0:w, 2 * fc:2 * fc + 2] = W2[:, fc * 128:fc * 128 + w].T

    shared = {
        "w1h": w1h,
        "wc1p": wc1p.astype(BF16),
        "wc0p": wc0p.astype(BF16),
        "b1r": b1.reshape(1, F1).astype(BF16),
        "w2tp": w2tp.astype(BF16),
        "b2c": b2.reshape(OUT, 1).astype(np.float32),
    }
    return [{"xh": xh[d], **shared} for d in range(NCORES)]


def _make_runner(nc):
    """Mirror bass2jax.run_bass_via_pjrt's multi-core path, but return a
    reusable jitted callable + input metadata so repeated executions don't
    retrace/retransfer (needed for HW timing: no NTFF profiling via axon
    in this container)."""
    rkey = ("runner", id(nc))
    if rkey in _CACHE:
        return _CACHE[rkey]

    import jax
    import concourse.mybir as mybir
    from jax.sharding import Mesh, PartitionSpec
    from jax.experimental.shard_map import shard_map
    from concourse import bass2jax

    bass2jax.install_neuronx_cc_hook()

    partition_name = (nc.partition_id_tensor.name
                      if nc.partition_id_tensor else None)
    in_names, out_names, out_avals = [], [], []
    for alloc in nc.m.functions[0].allocations:
        if not isinstance(alloc, mybir.MemoryLocationSet):
            continue
        name = alloc.memorylocations[0].name
        if alloc.kind == "ExternalInput":
            if name != partition_name:
                in_names.append(name)
        elif alloc.kind == "ExternalOutput":
            out_names.append(name)
            out_avals.append(jax.core.ShapedArray(
                tuple(alloc.tensor_shape), mybir.dt.np(alloc.dtype)))
    n_params = len(in_names)
    all_in_names = in_names + out_names
    if partition_name is not None:
        all_in_names.append(partition_name)
    donate = tuple(range(n_params, n_params + len(out_names)))

    def _body(*args):
        operands = list(args)
        if partition_name is not None:
            operands.append(bass2jax.partition_id_tensor())
        outs = bass2jax._bass_exec_p.bind(
            *operands,
            out_avals=tuple(out_avals),
            in_names=tuple(all_in_names),
            out_names=tuple(out_names),
            lowering_input_output_aliases=(),
            sim_require_finite=True,
            sim_require_nnan=True,
            nc=nc,
        )
        return tuple(outs)

    devices = jax.devices()[:NCORES]
    mesh = Mesh(np.asarray(devices), ("core",))
    spec = PartitionSpec("core")
    in_specs = (spec,) * (n_params + len(out_names))
    out_specs = (spec,) * len(out_names)
    fn = jax.jit(
        shard_map(_body, mesh=mesh, in_specs=in_specs, out_specs=out_specs,
                  check_rep=False),
        donate_argnums=donate, keep_unused=True,
    )
    runner = dict(fn=fn, mesh=mesh, spec=spec, in_names=in_names,
                  out_names=out_names, out_avals=out_avals)
    _CACHE[rkey] = runner
    return runner


def _stage_inputs(runner, in_maps):
    """Concatenate per-core inputs and put them device-resident, sharded."""
    import jax
    from jax.sharding import NamedSharding

    sharding = NamedSharding(runner["mesh"], runner["spec"])
    staged = []
    for name in runner["in_names"]:
        concat = np.concatenate([np.asarray(m[name]) for m in in_maps], axis=0)
        staged.append(jax.device_put(concat, sharding))
    return staged


def _zero_outs(runner):
    import jax
    from jax.sharding import NamedSharding

    sharding = NamedSharding(runner["mesh"], runner["spec"])
    return [
        jax.device_put(
            np.zeros((NCORES * a.shape[0], *a.shape[1:]), a.dtype), sharding)
        for a in runner["out_avals"]
    ]


def _assemble(runner, out_arrs):
    out_map = dict(zip(runner["out_names"], out_arrs))
    o = np.asarray(out_map["o"]).reshape(NCORES, OUT, BC)
    out = np.empty((B, OUT), dtype=np.float32)
    for d in range(NCORES):
        out[d * BC:(d + 1) * BC, :] = o[d].T
    return out


def run(inputs):
    nc = _build_nc()
    runner = _make_runner(nc)
    in_maps = _host_prep(**inputs)
    staged = _stage_inputs(runner, in_maps)
    out_arrs = runner["fn"](*staged, *_zero_outs(runner))
    return _assemble(runner, out_arrs)


def bench(inputs, iters=20):
    """Returns (output, per-iteration exec time ns) with inputs device-resident."""
    import time
    import jax

    nc = _build_nc()
    runner = _make_runner(nc)
    in_maps = _host_prep(**inputs)
    staged = _stage_inputs(runner, in_maps)

    # warmup (also the correctness output)
    out_arrs = runner["fn"](*staged, *_zero_outs(runner))
    jax.block_until_ready(out_arrs)
    out = _assemble(runner, out_arrs)

    zero_sets = [_zero_outs(runner) for _ in range(iters)]
    t0 = time.perf_counter()
    last = None
    for z in zero_sets:
        last = runner["fn"](*staged, *z)
    jax.block_until_ready(last)
    t1 = time.perf_counter()
    return out, (t1 - t0) / iters * 1e9


def kernel(**inputs) -> np.ndarray:
    return run(inputs)

